# revision 1
# baseline (speedup 1.0000x reference)
"""Trainium2 Bass kernel for DGLGraphConv-style message passing.

  m = feat[src] * edge_w[:, None]; h = segment_sum(m, dst, N); out = h @ W + b

Strategy (8 NeuronCores, SPMD, no collectives):
  * Host sorts edges by dst; each core owns a 12500-node dst range and all
    its in-edges. Nodes are degree-sorted into 128-node tiles; tiles are
    grouped into chunks.
  * The feat gather uses the DMAGatherAnt extended instruction (int16
    indices -> 32768-row limit), so each core's feat copy is row-permuted
    into 4 windows of 25000 rows. A greedy per-core assignment places each
    src node in the window that keeps every dst node's per-window edge
    counts balanced, minimizing ELL padding.
  * Per (chunk, window): slots are ELL-packed with chunk-uniform width
    S_cw; one dma_gather (split to <=5120 idxs) pulls 128B bf16 feat rows
    from the 256B-strided padded table.
  * Per chunk: one in-place DVE multiply by edge_w, per-window binary-tree
    slot reduction (bf16->f32 on level 1), window combine, then per tile:
    PE transpose + linear, DVE bias add, one batched row-scatter per chunk.
"""

import numpy as np
import ml_dtypes

N_NODES = 100000
N_EDGES = 1600000
D = 64
C = 8
P = 128
NPC = N_NODES // C            # 12500 nodes per core
T = (NPC + P - 1) // P        # 98 tiles per core
NPAD = T * P                  # 12544
W = 4                         # feat windows
NW = N_NODES // W             # 25000 rows per window
B_COLS = 192                  # max slot-columns per chunk
G_COLS = 40                   # max columns per sub-gather (5120 idxs)

_cache = {}


def _greedy_windows(src_c, dst_c):
    """Assign each src node to a window, balancing per-dst window counts.

    Returns pos[node] = row in this core's permuted feat table.
    """
    mult = np.bincount(src_c, minlength=N_NODES)
    cnt = np.zeros((NPC, W), dtype=np.int32)     # per-dst per-window counts
    cap = np.full(W, NW, dtype=np.int64)
    win = np.full(N_NODES, -1, dtype=np.int8)

    # edges grouped by src
    order = np.argsort(src_c, kind="stable")
    s_sorted = src_c[order]
    d_sorted = dst_c[order]
    starts = np.searchsorted(s_sorted, np.arange(N_NODES))
    ends = np.searchsorted(s_sorted, np.arange(N_NODES) + 1)

    active = np.nonzero(mult > 0)[0]
    # high-multiplicity srcs first: they constrain the most
    for s in active[np.argsort(-mult[active], kind="stable")]:
        dsts = d_sorted[starts[s]:ends[s]]
        sub = cnt[dsts]                           # [k, W]
        # pick window minimizing resulting max count, tie-break on load
        score = sub.max(axis=0) * 1000000 + (NW - cap)
        score[cap <= 0] = 1 << 60
        w = int(np.argmin(score))
        win[s] = w
        cnt[dsts, w] += 1
        cap[w] -= 1
    # nodes with no edges fill leftover capacity
    rest = np.nonzero(win < 0)[0]
    fill = np.repeat(np.arange(W), cap.astype(np.int64))
    win[rest] = fill[:len(rest)]

    pos = np.empty(N_NODES, dtype=np.int64)
    for w in range(W):
        nodes = np.nonzero(win == w)[0]
        pos[nodes] = w * NW + np.arange(len(nodes))
    return pos


def _build_tables(src, dst, edge_w):
    order = np.argsort(dst, kind="stable")
    src_s = src[order]
    dst_s = dst[order]
    w_s = edge_w[order]
    counts = np.bincount(dst, minlength=N_NODES)
    rowptr = np.zeros(N_NODES + 1, dtype=np.int64)
    np.cumsum(counts, out=rowptr[1:])

    # pass 1 per core: window assignment + degree sort + per-node counts
    percore = []
    tilemax = np.zeros((T, W), dtype=np.int64)
    for c in range(C):
        lo = c * NPC
        e0, e1 = rowptr[lo], rowptr[lo + NPC]
        src_c = src_s[e0:e1]
        dst_c = dst_s[e0:e1] - lo
        ew_c = w_s[e0:e1]
        deg = counts[lo:lo + NPC]

        pos = _greedy_windows(src_c, dst_c)
        psrc = pos[src_c]                        # permuted src row ids
        pwin = (psrc // NW).astype(np.int64)

        degp = np.concatenate([deg, np.full(NPAD - NPC, -1, dtype=deg.dtype)])
        nperm = np.argsort(-degp, kind="stable")

        cnt = np.zeros((NPC, W), dtype=np.int32)
        np.add.at(cnt, (dst_c, pwin), 1)
        cntp = np.vstack([cnt, np.zeros((NPAD - NPC, W), np.int32)])[nperm]
        tilemax = np.maximum(tilemax, cntp.reshape(T, P, W).max(axis=1))

        eorder = np.lexsort((pwin, dst_c))
        d_e = dst_c[eorder]
        ps_e = psrc[eorder]
        ew_e = ew_c[eorder]
        nstart = np.zeros(NPC + 1, dtype=np.int64)
        np.cumsum(np.bincount(d_e, minlength=NPC), out=nstart[1:])
        wstart = np.zeros((NPC, W + 1), dtype=np.int64)
        for w in range(W):
            wstart[:, w + 1] = wstart[:, w] + cnt[:, w]
        percore.append((nperm, cnt, nstart, wstart, ps_e, ew_e, pos))

    # shared schedule: greedy chunks over tiles using cross-core maxes
    chunks = []          # (t0, t1, [S_c0..S_c3], off_cols)
    t = 0
    off = 0
    while t < T:
        t0 = t
        Sw = np.zeros(W, dtype=np.int64)
        while t < T:
            S2 = np.maximum(Sw, tilemax[t])
            S2 = S2 + (S2 % 2)
            S2[0] = max(S2[0], 2)
            if (t - t0 + 1) * S2.sum() > B_COLS and t > t0:
                break
            Sw = S2
            t += 1
            if (t - t0) * Sw.sum() >= B_COLS:
                break
        chunks.append((t0, t, Sw.copy(), off))
        off += (t - t0) * int(Sw.sum())
    SUMS = off

    cores = []
    for c in range(C):
        nperm, cnt, nstart, wstart, ps_e, ew_e, pos = percore[c]
        # build idx (wrapped int16) + ew tables in region order
        idx_all = np.zeros((P, 8 * SUMS), dtype=np.int16)
        ew_all = np.zeros((P, SUMS), dtype=ml_dtypes.bfloat16)
        for (t0, t1, Sw, coff) in chunks:
            ntile = t1 - t0
            roff = 0
            for w in range(W):
                S = int(Sw[w])
                if S == 0:
                    continue
                cols = ntile * S
                # idx list position i -> (partition i%128, col i//128),
                # col j -> (tile j//S, slot j%S)
                nodes = nperm[t0 * P:(t1) * P]         # [ntile*128]
                real = nodes < NPC
                nn = np.minimum(nodes, NPC - 1)
                c_w = np.where(real, cnt[nn, w], 0)    # [ntile*128]
                base_e = nstart[nn] + wstart[nn, w]
                k = np.arange(S)
                # [ntile*128, S]
                valid = k[None, :] < c_w[:, None]
                e = base_e[:, None] + np.minimum(
                    k[None, :], np.maximum(c_w[:, None] - 1, 0))
                ividx = np.where(valid, ps_e[e] - w * NW, 0).astype(np.int16)
                ivew = np.where(valid, ew_e[e], 0.0)
                # reshape [ntile, 128, S] -> columns [ntile*S] of 128
                ividx = ividx.reshape(ntile, P, S).transpose(0, 2, 1)
                ivew = ivew.reshape(ntile, P, S).transpose(0, 2, 1)
                flat = ividx.reshape(cols * P)         # idx-list order
                wrapped = flat.reshape(cols * P // 16, 16).T   # [16, F]
                cs = coff + roff
                for g in range(8):
                    idx_all[g * 16:(g + 1) * 16, 8 * cs:8 * (cs + cols)] = \
                        wrapped
                # ew in msgs layout: [P, cols] where col j = (tile, slot)
                ew_all[:, cs:cs + cols] = np.ascontiguousarray(
                    ivew.transpose(2, 0, 1).reshape(P, cols)
                ).astype(ml_dtypes.bfloat16)
                roff += cols

        cores.append((idx_all, ew_all, nperm, pos))
    return chunks, SUMS, cores


def _emit_gather(gp, mybir, out_ap, in_ap, idxs_ap, num_idxs):
    """dma_gather emit without the 256B elem restriction (128B elems)."""
    from concourse.bass import exact_div
    stride_bytes = 2 * D * 2                         # 256B row stride
    return gp.add_instruction(
        mybir.InstDMAGatherAnt(
            name=gp.bass.get_next_instruction_name(),
            ins=[*gp.lower_ap_dma(in_ap, for_custom_bir_dma=True),
                 gp.lower_ap(idxs_ap),
                 gp.lower_val_access(gp.to_reg(num_idxs))],
            outs=[gp.lower_ap(out_ap)],
            transpose=False,
            num_idxs=num_idxs,
            elem_size=D,
            stride_bytes_256=exact_div(stride_bytes, 256),
            gen_mode=0,
            single_packet=False,
            queue_num=0,
            sbuf_tokens_per_rank=0,
            sbuf_free_dim_per_rank=0,
            sbuf_free_dim_pad_per_rank=0,
            sbuf_byte_offset=0,
        ))


def _build_program(chunks, SUMS):
    import concourse.bass as bass
    import concourse.mybir as mybir
    import concourse.tile as tile
    from concourse import bacc
    from concourse.masks import make_identity

    f32 = mybir.dt.float32
    bf16 = mybir.dt.bfloat16
    i16 = mybir.dt.int16

    nc = bacc.Bacc("TRN2", target_bir_lowering=False, debug=False,
                   num_devices=C, dynamic_dma_scratch_size=49152)

    featb = nc.dram_tensor("featb", [N_NODES, 2 * D], bf16,
                           kind="ExternalInput").ap()
    idx_d = nc.dram_tensor("idx", [P, 8 * SUMS], i16,
                           kind="ExternalInput").ap()
    ew_d = nc.dram_tensor("ew", [P, SUMS], bf16, kind="ExternalInput").ap()
    wt_d = nc.dram_tensor("wt", [D, D], f32, kind="ExternalInput").ap()
    brep_d = nc.dram_tensor("brep", [P, D], f32, kind="ExternalInput").ap()
    out_d = nc.dram_tensor("out", [NPAD, D], f32, kind="ExternalOutput").ap()

    with tile.TileContext(nc) as tc:
        with (
            tc.tile_pool(name="const", bufs=1) as cpool,
            tc.tile_pool(name="sb", bufs=2) as pool,
            tc.tile_pool(name="ps", bufs=2, space="PSUM") as pspool,
        ):
            W_sb = cpool.tile([D, D], f32)
            nc.sync.dma_start(W_sb[:], wt_d[:])
            brep_sb = cpool.tile([P, D], f32)
            nc.sync.dma_start(brep_sb[:], brep_d[:])
            ident = cpool.tile([P, P], f32)
            make_identity(nc, ident[:])

            for (t0, t1, Sw, coff) in chunks:
                ntile = t1 - t0
                ccols = ntile * int(Sw.sum())

                idx_sb = pool.tile([P, 8 * ccols], i16, tag="idx")
                nc.sync.dma_start(idx_sb[:],
                                  idx_d[:, 8 * coff:8 * (coff + ccols)])
                ew_sb = pool.tile([P, ccols], bf16, tag="ew")
                nc.sync.dma_start(ew_sb[:], ew_d[:, coff:coff + ccols])

                msgs = pool.tile([P, ccols * D], bf16, tag="msgs")
                roff = 0
                for w in range(W):
                    S = int(Sw[w])
                    if S == 0:
                        continue
                    cols = ntile * S
                    fw = featb[w * NW:(w + 1) * NW, :]
                    for g0 in range(0, cols, G_COLS):
                        gc = min(G_COLS, cols - g0)
                        o0 = roff + g0
                        _emit_gather(
                            nc.gpsimd, mybir,
                            out_ap=msgs[:, o0 * D:(o0 + gc) * D].rearrange(
                                "p (c d) -> p c d", d=D),
                            in_ap=fw,
                            idxs_ap=idx_sb[:, 8 * o0:8 * (o0 + gc)],
                            num_idxs=gc * P,
                        )
                    roff += cols

                # in-place multiply by edge weight
                m3 = msgs[:].rearrange("p (s d) -> p s d", d=D)
                ewb = ew_sb[:].unsqueeze(2).to_broadcast([P, ccols, D])
                nc.vector.tensor_tensor(out=m3, in0=m3, in1=ewb,
                                        op=mybir.AluOpType.mult)

                # per-window chunk-wide trees; level 1 casts to f32
                h1s = [max(int(Sw[w]) // 2, 1) if int(Sw[w]) > 0 else 0
                       for w in range(W)]
                acc = pool.tile([P, ntile * sum(h1s) * D], f32, tag="acc")
                hviews = []
                roff = 0
                aoff = 0
                for w in range(W):
                    S = int(Sw[w])
                    if S == 0:
                        continue
                    h1 = h1s[w]
                    w4 = msgs[:, roff * D:(roff + ntile * S) * D].rearrange(
                        "p (t s d) -> p t s d", t=ntile, d=D)
                    a4 = acc[:, aoff * D:(aoff + ntile * h1) * D].rearrange(
                        "p (t s d) -> p t s d", t=ntile, d=D)
                    if S == 1:
                        nc.vector.tensor_scalar(
                            out=a4[:, :, 0, :], in0=w4[:, :, 0, :],
                            scalar1=1.0, scalar2=None,
                            op0=mybir.AluOpType.mult)
                    else:
                        nc.vector.tensor_tensor(
                            out=a4, in0=w4[:, :, 0:h1, :],
                            in1=w4[:, :, h1:2 * h1, :],
                            op=mybir.AluOpType.add)
                        if S % 2 == 1:
                            nc.vector.tensor_tensor(
                                out=a4[:, :, 0, :], in0=a4[:, :, 0, :],
                                in1=w4[:, :, S - 1, :],
                                op=mybir.AluOpType.add)
                    cur = h1
                    while cur > 1:
                        h = cur // 2
                        nc.vector.tensor_tensor(
                            out=a4[:, :, 0:h, :], in0=a4[:, :, 0:h, :],
                            in1=a4[:, :, cur - h:cur, :],
                            op=mybir.AluOpType.add)
                        cur = h + (cur - 2 * h)
                    hviews.append(a4[:, :, 0, :])      # [P, ntile, D]
                    roff += ntile * S
                    aoff += ntile * h1

                houtc = pool.tile([P, ntile * D], f32, tag="houtc")
                hc3 = houtc[:].rearrange("p (t d) -> p t d", d=D)
                if len(hviews) == 1:
                    nc.vector.tensor_scalar(
                        out=hc3, in0=hviews[0], scalar1=1.0, scalar2=None,
                        op0=mybir.AluOpType.mult)
                else:
                    nc.vector.tensor_tensor(out=hc3, in0=hviews[0],
                                            in1=hviews[1],
                                            op=mybir.AluOpType.add)
                    for hv in hviews[2:]:
                        nc.vector.tensor_tensor(out=hc3, in0=hc3, in1=hv,
                                                op=mybir.AluOpType.add)

                outc = pool.tile([P, ntile * D], f32, tag="outc")
                for j in range(ntile):
                    hj = houtc[:, j * D:(j + 1) * D]
                    pt = pspool.tile([D, P], f32, tag="pt")
                    nc.tensor.transpose(pt[:], hj, ident[:])
                    ht = pool.tile([D, P], f32, tag="ht")
                    nc.scalar.copy(ht[:], pt[:])
                    po = pspool.tile([P, D], f32, tag="po")
                    nc.tensor.matmul(po[:], lhsT=ht[:], rhs=W_sb[:],
                                     start=True, stop=True)
                    nc.vector.tensor_tensor(
                        out=outc[:, j * D:(j + 1) * D], in0=po[:],
                        in1=brep_sb[:], op=mybir.AluOpType.add)

                od = out_d[t0 * P:t1 * P, :].rearrange("(t p) d -> p t d", p=P)
                nc.sync.dma_start(od, outc[:].rearrange("p (t d) -> p t d",
                                                        d=D))

    nc.compile()
    return nc


def _prepare(feat, edge_w, src, dst, weight, bias):
    feat = np.asarray(feat, dtype=np.float32)
    edge_w = np.asarray(edge_w, dtype=np.float32)
    src = np.asarray(src, dtype=np.int32)
    dst = np.asarray(dst, dtype=np.int32)
    weight = np.asarray(weight, dtype=np.float32)
    bias = np.asarray(bias, dtype=np.float32)

    chunks, SUMS, cores = _build_tables(src, dst, edge_w)
    brep = np.ascontiguousarray(np.broadcast_to(bias, (P, D))).astype(
        np.float32)

    feat_pad = np.zeros((N_NODES, 2 * D), dtype=ml_dtypes.bfloat16)
    feat_pad[:, :D] = feat.astype(ml_dtypes.bfloat16)

    in_maps = []
    perms = []
    for c in range(C):
        idx_all, ew_all, nperm, pos = cores[c]
        fp = np.empty_like(feat_pad)
        fp[pos] = feat_pad                       # row r of fp = feat[inv]
        in_maps.append({
            "featb": fp,
            "idx": idx_all,
            "ew": ew_all,
            "wt": weight,
            "brep": brep,
        })
        perms.append(nperm)
    return chunks, SUMS, in_maps, perms


def kernel(feat, edge_w, src, dst, weight, bias, _trace=False):
    chunks, SUMS, in_maps, perms = _prepare(
        feat, edge_w, src, dst, weight, bias)

    key = tuple((t0, t1, tuple(Sw)) for (t0, t1, Sw, _) in chunks)
    if key not in _cache:
        _cache[key] = _build_program(chunks, SUMS)
    nc = _cache[key]

    from concourse.bass_utils import run_bass_kernel_spmd
    res = run_bass_kernel_spmd(nc, in_maps, core_ids=list(range(C)),
                               trace=_trace)
    out = np.empty((N_NODES, D), dtype=np.float32)
    for c in range(C):
        o = np.asarray(res.results[c]["out"])
        nperm = perms[c]
        real = nperm < NPC
        out[c * NPC + nperm[real]] = o[np.nonzero(real)[0]]
    if _trace:
        kernel.last_results = res
    return out



# revision 2
# speedup vs baseline: 2.6368x; 2.6368x over previous
"""Trainium2 Bass kernel for DGLGraphConv-style message passing.

  m = feat[src] * edge_w[:, None]; h = segment_sum(m, dst, N); out = h @ W + b

Strategy (8 NeuronCores, SPMD, no collectives):
  * Host sorts edges by dst; each core owns a 12500-node dst range and all
    its in-edges. Nodes are degree-sorted into 128-node tiles; tiles are
    grouped into chunks with chunk-uniform ELL slot width per window.
  * The feat gather uses the DMAGatherAnt extended instruction (int16
    indices -> 32768-row limit), so each core's feat copy is row-permuted
    into 4 windows of 25000 rows. A greedy per-core assignment places each
    src node in the window that keeps every dst node's per-window edge
    counts balanced, minimizing ELL padding.
  * Gathers are issued round-robin on 4 SWDGE queues, so descriptor
    generation runs on all 4 GPSIMD core pairs concurrently (~4x).
    Each gather's int16 index stream lives only in its queue's
    32-partition band (2 copies, one per Q7 core of the pair).
  * Chunk columns are slot-major (col = s*ntile + t), so the per-window
    slot-tree reduction uses fully contiguous DVE operands.
  * Per chunk: one in-place DVE multiply by edge_w, per-window binary-tree
    slot reduction (bf16->f32 on level 1), window combine, then per tile:
    PE transpose + linear, DVE bias add, one batched row-scatter per chunk.
"""

import numpy as np
import ml_dtypes

N_NODES = 100000
N_EDGES = 1600000
D = 64
C = 8
P = 128
NPC = N_NODES // C            # 12500 nodes per core
T = (NPC + P - 1) // P        # 98 tiles per core
NPAD = T * P                  # 12544
W = 4                         # feat windows
NW = N_NODES // W             # 25000 rows per window
B_COLS = 160                  # max slot-columns per chunk
G_COLS = 40                   # max columns per gather (5120 idxs)
NQ = 4                        # SWDGE queues

_cache = {}


def _greedy_windows(src_c, dst_c):
    """Assign each src node to a window, balancing per-dst window counts.

    Returns pos[node] = row in this core's permuted feat table.
    """
    mult = np.bincount(src_c, minlength=N_NODES)
    cnt = np.zeros((NPC, W), dtype=np.int32)     # per-dst per-window counts
    cap = np.full(W, NW, dtype=np.int64)
    win = np.full(N_NODES, -1, dtype=np.int8)

    # edges grouped by src
    order = np.argsort(src_c, kind="stable")
    s_sorted = src_c[order]
    d_sorted = dst_c[order]
    starts = np.searchsorted(s_sorted, np.arange(N_NODES))
    ends = np.searchsorted(s_sorted, np.arange(N_NODES) + 1)

    active = np.nonzero(mult > 0)[0]
    # high-multiplicity srcs first: they constrain the most
    for s in active[np.argsort(-mult[active], kind="stable")]:
        dsts = d_sorted[starts[s]:ends[s]]
        sub = cnt[dsts]                           # [k, W]
        # pick window minimizing resulting max count, tie-break on load
        score = sub.max(axis=0) * 1000000 + (NW - cap)
        score[cap <= 0] = 1 << 60
        w = int(np.argmin(score))
        win[s] = w
        cnt[dsts, w] += 1
        cap[w] -= 1
    # nodes with no edges fill leftover capacity
    rest = np.nonzero(win < 0)[0]
    fill = np.repeat(np.arange(W), cap.astype(np.int64))
    win[rest] = fill[:len(rest)]

    pos = np.empty(N_NODES, dtype=np.int64)
    for w in range(W):
        nodes = np.nonzero(win == w)[0]
        pos[nodes] = w * NW + np.arange(len(nodes))
    return pos


def _make_schedule(tilemax):
    """Greedy chunks over tiles using cross-core maxes.

    Returns chunks [(t0, t1, Sw, col_off)], total columns, and the shared
    gather plan: per chunk a list of (window, c0, gc, queue, band_off),
    plus per-chunk idx-tile free sizes and offsets.
    """
    chunks = []
    t = 0
    off = 0
    while t < T:
        t0 = t
        Sw = np.zeros(W, dtype=np.int64)
        while t < T:
            S2 = np.maximum(Sw, tilemax[t])
            if (t - t0 + 1) * S2.sum() > B_COLS and t > t0:
                break
            Sw = S2
            t += 1
            if (t - t0) * Sw.sum() >= B_COLS:
                break
        chunks.append((t0, t, Sw.copy(), off))
        off += (t - t0) * int(Sw.sum())
    SUMS = off

    # gather plan: round-robin queues, per-chunk per-queue band offsets
    plan = []          # per chunk: list of (w, c0_in_window, gc, q, boff)
    fsizes = []        # per chunk: idx tile free size (int16 elems)
    foffs = []         # per chunk: offset in dram idx table
    qctr = 0
    ftot = 0
    for (t0, t1, Sw, coff) in chunks:
        ntile = t1 - t0
        gaths = []
        boff = [0] * NQ
        for w in range(W):
            S = int(Sw[w])
            if S == 0:
                continue
            cols = ntile * S
            for c0 in range(0, cols, G_COLS):
                gc = min(G_COLS, cols - c0)
                q = qctr % NQ
                qctr += 1
                gaths.append((w, c0, gc, q, boff[q]))
                boff[q] += gc * 8          # F elems per gather
        fmax = max(boff) if boff else 0
        plan.append(gaths)
        foffs.append(ftot)
        fsizes.append(fmax)
        ftot += fmax
    return chunks, SUMS, plan, fsizes, foffs, ftot


def _build_tables(src, dst, edge_w):
    order = np.argsort(dst, kind="stable")
    src_s = src[order]
    dst_s = dst[order]
    w_s = edge_w[order]
    counts = np.bincount(dst, minlength=N_NODES)
    rowptr = np.zeros(N_NODES + 1, dtype=np.int64)
    np.cumsum(counts, out=rowptr[1:])

    # pass 1 per core: window assignment + degree sort + per-node counts
    percore = []
    tilemax = np.zeros((T, W), dtype=np.int64)
    for c in range(C):
        lo = c * NPC
        e0, e1 = rowptr[lo], rowptr[lo + NPC]
        src_c = src_s[e0:e1]
        dst_c = dst_s[e0:e1] - lo
        ew_c = w_s[e0:e1]
        deg = counts[lo:lo + NPC]

        pos = _greedy_windows(src_c, dst_c)
        psrc = pos[src_c]                        # permuted src row ids
        pwin = (psrc // NW).astype(np.int64)

        degp = np.concatenate([deg, np.full(NPAD - NPC, -1, dtype=deg.dtype)])
        nperm = np.argsort(-degp, kind="stable")

        cnt = np.zeros((NPC, W), dtype=np.int32)
        np.add.at(cnt, (dst_c, pwin), 1)
        cntp = np.vstack([cnt, np.zeros((NPAD - NPC, W), np.int32)])[nperm]
        tilemax = np.maximum(tilemax, cntp.reshape(T, P, W).max(axis=1))

        eorder = np.lexsort((pwin, dst_c))
        d_e = dst_c[eorder]
        ps_e = psrc[eorder]
        ew_e = ew_c[eorder]
        nstart = np.zeros(NPC + 1, dtype=np.int64)
        np.cumsum(np.bincount(d_e, minlength=NPC), out=nstart[1:])
        wstart = np.zeros((NPC, W + 1), dtype=np.int64)
        for w in range(W):
            wstart[:, w + 1] = wstart[:, w] + cnt[:, w]
        percore.append((nperm, cnt, nstart, wstart, ps_e, ew_e, pos))

    chunks, SUMS, plan, fsizes, foffs, FTOT = _make_schedule(tilemax)

    cores = []
    for c in range(C):
        nperm, cnt, nstart, wstart, ps_e, ew_e, pos = percore[c]
        idx_all = np.zeros((P, FTOT), dtype=np.int16)
        ew_all = np.zeros((P, SUMS), dtype=ml_dtypes.bfloat16)
        for ci, (t0, t1, Sw, coff) in enumerate(chunks):
            ntile = t1 - t0
            # per-window ELL entries [ntile*128, S] in slot-major column
            # order: window-region col j = s*ntile + t
            roff = 0
            wdat = {}
            for w in range(W):
                S = int(Sw[w])
                if S == 0:
                    continue
                nodes = nperm[t0 * P:t1 * P]           # [ntile*128]
                real = nodes < NPC
                nn = np.minimum(nodes, NPC - 1)
                c_w = np.where(real, cnt[nn, w], 0)
                base_e = nstart[nn] + wstart[nn, w]
                k = np.arange(S)
                valid = k[None, :] < c_w[:, None]
                e = base_e[:, None] + np.minimum(
                    k[None, :], np.maximum(c_w[:, None] - 1, 0))
                ividx = np.where(valid, ps_e[e] - w * NW, 0).astype(np.int16)
                ivew = np.where(valid, ew_e[e], 0.0)
                # [ntile, 128, S] -> columns [S, ntile] of 128 (slot-major)
                ividx = ividx.reshape(ntile, P, S).transpose(2, 0, 1)
                ivew = ivew.reshape(ntile, P, S).transpose(2, 0, 1)
                ividx = ividx.reshape(S * ntile, P)    # [cols, 128]
                ivew = ivew.reshape(S * ntile, P)
                wdat[w] = ividx
                cols = S * ntile
                ew_all[:, coff + roff:coff + roff + cols] = \
                    ivew.T.astype(ml_dtypes.bfloat16)
                roff += cols
            # idx streams per gather, banded by queue
            for (w, c0, gc, q, boff) in plan[ci]:
                ividx = wdat[w]
                flat = ividx[c0:c0 + gc].reshape(gc * P)   # list order
                wrapped = flat.reshape(gc * P // 16, 16).T  # [16, F/...]
                f0 = foffs[ci] + boff
                F = gc * 8
                idx_all[32 * q:32 * q + 16, f0:f0 + F] = wrapped
                idx_all[32 * q + 16:32 * q + 32, f0:f0 + F] = wrapped

        cores.append((idx_all, ew_all, nperm, pos))
    return chunks, SUMS, plan, fsizes, foffs, FTOT, cores


def _emit_gather(gp, mybir, out_ap, in_ap, idxs_ap, num_idxs, queue_num):
    """dma_gather emit without the 256B elem restriction (128B elems)."""
    from concourse.bass import exact_div
    stride_bytes = 2 * D * 2                         # 256B row stride
    return gp.add_instruction(
        mybir.InstDMAGatherAnt(
            name=gp.bass.get_next_instruction_name(),
            ins=[*gp.lower_ap_dma(in_ap, for_custom_bir_dma=True),
                 gp.lower_ap(idxs_ap),
                 gp.lower_val_access(gp.to_reg(num_idxs))],
            outs=[gp.lower_ap(out_ap)],
            transpose=False,
            num_idxs=num_idxs,
            elem_size=D,
            stride_bytes_256=exact_div(stride_bytes, 256),
            gen_mode=0,
            single_packet=False,
            queue_num=queue_num,
            sbuf_tokens_per_rank=0,
            sbuf_free_dim_per_rank=0,
            sbuf_free_dim_pad_per_rank=0,
            sbuf_byte_offset=0,
        ))


def _build_program(chunks, SUMS, plan, fsizes, foffs, FTOT):
    import concourse.bass as bass
    import concourse.mybir as mybir
    import concourse.tile as tile
    from concourse import bacc
    from concourse.masks import make_identity

    f32 = mybir.dt.float32
    bf16 = mybir.dt.bfloat16
    i16 = mybir.dt.int16

    nc = bacc.Bacc("TRN2", target_bir_lowering=False, debug=False,
                   num_devices=C, dynamic_dma_scratch_size=49152,
                   num_swdge_queues=NQ)

    featb = nc.dram_tensor("featb", [N_NODES, 2 * D], bf16,
                           kind="ExternalInput").ap()
    idx_d = nc.dram_tensor("idx", [P, FTOT], i16, kind="ExternalInput").ap()
    ew_d = nc.dram_tensor("ew", [P, SUMS], bf16, kind="ExternalInput").ap()
    wt_d = nc.dram_tensor("wt", [D, D], f32, kind="ExternalInput").ap()
    brep_d = nc.dram_tensor("brep", [P, D], f32, kind="ExternalInput").ap()
    out_d = nc.dram_tensor("out", [NPAD, D], f32, kind="ExternalOutput").ap()

    with tile.TileContext(nc) as tc:
        with (
            tc.tile_pool(name="const", bufs=1) as cpool,
            tc.tile_pool(name="sb", bufs=2) as pool,
            tc.tile_pool(name="ps", bufs=2, space="PSUM") as pspool,
        ):
            W_sb = cpool.tile([D, D], f32)
            nc.sync.dma_start(W_sb[:], wt_d[:])
            brep_sb = cpool.tile([P, D], f32)
            nc.sync.dma_start(brep_sb[:], brep_d[:])
            ident = cpool.tile([P, P], f32)
            make_identity(nc, ident[:])

            for ci, (t0, t1, Sw, coff) in enumerate(chunks):
                ntile = t1 - t0
                ccols = ntile * int(Sw.sum())
                fsz = max(fsizes[ci], 8)

                idx_sb = pool.tile([P, fsz], i16, tag="idx")
                nc.sync.dma_start(idx_sb[:],
                                  idx_d[:, foffs[ci]:foffs[ci] + fsz])
                ew_sb = pool.tile([P, ccols], bf16, tag="ew")
                nc.sync.dma_start(ew_sb[:], ew_d[:, coff:coff + ccols])

                msgs = pool.tile([P, ccols * D], bf16, tag="msgs")
                # window region offsets within chunk
                woff = {}
                roff = 0
                for w in range(W):
                    S = int(Sw[w])
                    if S == 0:
                        continue
                    woff[w] = roff
                    roff += ntile * S
                for (w, c0, gc, q, boff) in plan[ci]:
                    o0 = woff[w] + c0
                    fw = featb[w * NW:(w + 1) * NW, :]
                    _emit_gather(
                        nc.gpsimd, mybir,
                        out_ap=msgs[:, o0 * D:(o0 + gc) * D].rearrange(
                            "p (c d) -> p c d", d=D),
                        in_ap=fw,
                        idxs_ap=idx_sb[:, boff:boff + gc * 8],
                        num_idxs=gc * P,
                        queue_num=q,
                    )

                # in-place multiply by edge weight
                m3 = msgs[:].rearrange("p (s d) -> p s d", d=D)
                ewb = ew_sb[:].unsqueeze(2).to_broadcast([P, ccols, D])
                nc.vector.tensor_tensor(out=m3, in0=m3, in1=ewb,
                                        op=mybir.AluOpType.mult)

                # per-window contiguous slot trees; level 1 casts to f32
                h1s = {w: max(int(Sw[w]) // 2, 1)
                       for w in range(W) if int(Sw[w]) > 0}
                acc = pool.tile([P, ntile * sum(h1s.values()) * D], f32,
                                tag="acc")
                hviews = []
                aoff = 0
                for w in range(W):
                    S = int(Sw[w])
                    if S == 0:
                        continue
                    h1 = h1s[w]
                    nd = ntile * D
                    mw = msgs[:, woff[w] * D:(woff[w] + ntile * S) * D]
                    aw = acc[:, aoff * D:(aoff + ntile * h1) * D]
                    if S == 1:
                        nc.vector.tensor_scalar(
                            out=aw[:, 0:nd], in0=mw[:, 0:nd],
                            scalar1=1.0, scalar2=None,
                            op0=mybir.AluOpType.mult)
                    else:
                        nc.vector.tensor_tensor(
                            out=aw[:, 0:h1 * nd], in0=mw[:, 0:h1 * nd],
                            in1=mw[:, h1 * nd:2 * h1 * nd],
                            op=mybir.AluOpType.add)
                        if S % 2 == 1:
                            nc.vector.tensor_tensor(
                                out=aw[:, 0:nd], in0=aw[:, 0:nd],
                                in1=mw[:, (S - 1) * nd:S * nd],
                                op=mybir.AluOpType.add)
                    cur = h1
                    while cur > 1:
                        h = cur // 2
                        nc.vector.tensor_tensor(
                            out=aw[:, 0:h * nd], in0=aw[:, 0:h * nd],
                            in1=aw[:, (cur - h) * nd:cur * nd],
                            op=mybir.AluOpType.add)
                        cur = cur - h
                    hviews.append(aw[:, 0:nd])
                    aoff += ntile * h1

                houtc = pool.tile([P, ntile * D], f32, tag="houtc")
                if len(hviews) == 1:
                    nc.vector.tensor_scalar(
                        out=houtc[:], in0=hviews[0], scalar1=1.0,
                        scalar2=None, op0=mybir.AluOpType.mult)
                else:
                    nc.vector.tensor_tensor(out=houtc[:], in0=hviews[0],
                                            in1=hviews[1],
                                            op=mybir.AluOpType.add)
                    for hv in hviews[2:]:
                        nc.vector.tensor_tensor(out=houtc[:], in0=houtc[:],
                                                in1=hv,
                                                op=mybir.AluOpType.add)

                outc = pool.tile([P, ntile * D], f32, tag="outc")
                for j in range(ntile):
                    hj = houtc[:, j * D:(j + 1) * D]
                    pt = pspool.tile([D, P], f32, tag="pt")
                    nc.tensor.transpose(pt[:], hj, ident[:])
                    ht = pool.tile([D, P], f32, tag="ht")
                    nc.scalar.copy(ht[:], pt[:])
                    po = pspool.tile([P, D], f32, tag="po")
                    nc.tensor.matmul(po[:], lhsT=ht[:], rhs=W_sb[:],
                                     start=True, stop=True)
                    nc.vector.tensor_tensor(
                        out=outc[:, j * D:(j + 1) * D], in0=po[:],
                        in1=brep_sb[:], op=mybir.AluOpType.add)

                od = out_d[t0 * P:t1 * P, :].rearrange("(t p) d -> p t d", p=P)
                nc.sync.dma_start(od, outc[:].rearrange("p (t d) -> p t d",
                                                        d=D))

    nc.compile()
    return nc


def _prepare(feat, edge_w, src, dst, weight, bias):
    feat = np.asarray(feat, dtype=np.float32)
    edge_w = np.asarray(edge_w, dtype=np.float32)
    src = np.asarray(src, dtype=np.int32)
    dst = np.asarray(dst, dtype=np.int32)
    weight = np.asarray(weight, dtype=np.float32)
    bias = np.asarray(bias, dtype=np.float32)

    chunks, SUMS, plan, fsizes, foffs, FTOT, cores = _build_tables(
        src, dst, edge_w)
    brep = np.ascontiguousarray(np.broadcast_to(bias, (P, D))).astype(
        np.float32)

    feat_pad = np.zeros((N_NODES, 2 * D), dtype=ml_dtypes.bfloat16)
    feat_pad[:, :D] = feat.astype(ml_dtypes.bfloat16)

    in_maps = []
    perms = []
    for c in range(C):
        idx_all, ew_all, nperm, pos = cores[c]
        fp = np.empty_like(feat_pad)
        fp[pos] = feat_pad                       # row r of fp = feat[inv]
        in_maps.append({
            "featb": fp,
            "idx": idx_all,
            "ew": ew_all,
            "wt": weight,
            "brep": brep,
        })
        perms.append(nperm)
    return chunks, SUMS, plan, fsizes, foffs, FTOT, in_maps, perms


def kernel(feat, edge_w, src, dst, weight, bias, _trace=False):
    (chunks, SUMS, plan, fsizes, foffs, FTOT, in_maps,
     perms) = _prepare(feat, edge_w, src, dst, weight, bias)

    key = tuple((t0, t1, tuple(Sw)) for (t0, t1, Sw, _) in chunks)
    if key not in _cache:
        _cache[key] = _build_program(chunks, SUMS, plan, fsizes, foffs, FTOT)
    nc = _cache[key]

    from concourse.bass_utils import run_bass_kernel_spmd
    res = run_bass_kernel_spmd(nc, in_maps, core_ids=list(range(C)),
                               trace=_trace)
    out = np.empty((N_NODES, D), dtype=np.float32)
    for c in range(C):
        o = np.asarray(res.results[c]["out"])
        nperm = perms[c]
        real = nperm < NPC
        out[c * NPC + nperm[real]] = o[np.nonzero(real)[0]]
    if _trace:
        kernel.last_results = res
    return out


# revision 3
# speedup vs baseline: 3.3668x; 1.2769x over previous
"""Trainium2 Bass kernel for DGLGraphConv-style message passing.

  m = feat[src] * edge_w[:, None]; h = segment_sum(m, dst, N); out = h @ W + b

Strategy (8 NeuronCores, SPMD, no collectives):
  * Host sorts edges by dst; each core owns a 12500-node dst range and all
    its in-edges. Nodes are degree-sorted into 128-node tiles; tiles are
    grouped into chunks with chunk-uniform ELL slot width per window.
  * The feat gather uses the DMAGatherAnt extended instruction (int16
    indices -> 32768-row limit), so each core's feat copy is row-permuted
    into 4 windows of 25000 rows. A greedy per-core assignment places each
    src node in the window that keeps every dst node's per-window edge
    counts balanced, minimizing ELL padding.
  * Gathers are issued round-robin on 4 SWDGE queues, so descriptor
    generation runs on all 4 GPSIMD core pairs concurrently (~4x).
    Each gather's int16 index stream lives only in its queue's
    32-partition band (2 copies, one per Q7 core of the pair).
  * Chunk columns are slot-major (col = s*ntile + t), so the per-window
    slot-tree reduction uses fully contiguous DVE operands.
  * Per chunk: one in-place DVE multiply by edge_w, per-window binary-tree
    slot reduction (bf16->f32 on level 1), window combine, then per tile:
    PE transpose + linear, DVE bias add, one batched row-scatter per chunk.
"""

import numpy as np
import ml_dtypes

N_NODES = 100000
N_EDGES = 1600000
D = 64
C = 8
P = 128
NPC = N_NODES // C            # 12500 nodes per core
T = (NPC + P - 1) // P        # 98 tiles per core
NPAD = T * P                  # 12544
W = 4                         # feat windows
NW = N_NODES // W             # 25000 rows per window
B_COLS = 128                  # max slot-columns per chunk
G_COLS = 40                   # max columns per gather (5120 idxs)
NQ = 4                        # SWDGE queues

_cache = {}


def _greedy_windows(src_c, dst_c):
    """Assign each src node to a window, balancing per-dst window counts.

    Returns pos[node] = row in this core's permuted feat table.
    """
    mult = np.bincount(src_c, minlength=N_NODES)
    cnt = np.zeros((NPC, W), dtype=np.int32)     # per-dst per-window counts
    cap = np.full(W, NW, dtype=np.int64)
    win = np.full(N_NODES, -1, dtype=np.int8)

    # edges grouped by src
    order = np.argsort(src_c, kind="stable")
    s_sorted = src_c[order]
    d_sorted = dst_c[order]
    starts = np.searchsorted(s_sorted, np.arange(N_NODES))
    ends = np.searchsorted(s_sorted, np.arange(N_NODES) + 1)

    active = np.nonzero(mult > 0)[0]
    # high-multiplicity srcs first: they constrain the most
    for s in active[np.argsort(-mult[active], kind="stable")]:
        dsts = d_sorted[starts[s]:ends[s]]
        sub = cnt[dsts]                           # [k, W]
        # pick window minimizing resulting max count, tie-break on load
        score = sub.max(axis=0) * 1000000 + (NW - cap)
        score[cap <= 0] = 1 << 60
        w = int(np.argmin(score))
        win[s] = w
        cnt[dsts, w] += 1
        cap[w] -= 1
    # nodes with no edges fill leftover capacity
    rest = np.nonzero(win < 0)[0]
    fill = np.repeat(np.arange(W), cap.astype(np.int64))
    win[rest] = fill[:len(rest)]

    pos = np.empty(N_NODES, dtype=np.int64)
    for w in range(W):
        nodes = np.nonzero(win == w)[0]
        pos[nodes] = w * NW + np.arange(len(nodes))
    return pos


def _make_schedule(tilemax):
    """Greedy chunks over tiles using cross-core maxes.

    Returns chunks [(t0, t1, Sw, col_off)], total columns, and the shared
    gather plan: per chunk a list of (window, c0, gc, queue, band_off),
    plus per-chunk idx-tile free sizes and offsets.
    """
    chunks = []
    t = 0
    off = 0
    while t < T:
        t0 = t
        Sw = np.zeros(W, dtype=np.int64)
        while t < T:
            S2 = np.maximum(Sw, tilemax[t])
            if (t - t0 + 1) * S2.sum() > B_COLS and t > t0:
                break
            Sw = S2
            t += 1
            if (t - t0) * Sw.sum() >= B_COLS:
                break
        chunks.append((t0, t, Sw.copy(), off))
        off += (t - t0) * int(Sw.sum())
    SUMS = off

    # gather plan: round-robin queues, per-chunk per-queue band offsets
    plan = []          # per chunk: list of (w, c0_in_window, gc, q, boff)
    fsizes = []        # per chunk: idx tile free size (int16 elems)
    foffs = []         # per chunk: offset in dram idx table
    qctr = 0
    ftot = 0
    for (t0, t1, Sw, coff) in chunks:
        ntile = t1 - t0
        gaths = []
        boff = [0] * NQ
        for w in range(W):
            S = int(Sw[w])
            if S == 0:
                continue
            cols = ntile * S
            for c0 in range(0, cols, G_COLS):
                gc = min(G_COLS, cols - c0)
                q = qctr % NQ
                qctr += 1
                gaths.append((w, c0, gc, q, boff[q]))
                boff[q] += gc * 8          # F elems per gather
        fmax = max(boff) if boff else 0
        plan.append(gaths)
        foffs.append(ftot)
        fsizes.append(fmax)
        ftot += fmax
    return chunks, SUMS, plan, fsizes, foffs, ftot


def _build_tables(src, dst, edge_w):
    order = np.argsort(dst, kind="stable")
    src_s = src[order]
    dst_s = dst[order]
    w_s = edge_w[order]
    counts = np.bincount(dst, minlength=N_NODES)
    rowptr = np.zeros(N_NODES + 1, dtype=np.int64)
    np.cumsum(counts, out=rowptr[1:])

    # pass 1 per core: window assignment + degree sort + per-node counts
    percore = []
    tilemax = np.zeros((T, W), dtype=np.int64)
    for c in range(C):
        lo = c * NPC
        e0, e1 = rowptr[lo], rowptr[lo + NPC]
        src_c = src_s[e0:e1]
        dst_c = dst_s[e0:e1] - lo
        ew_c = w_s[e0:e1]
        deg = counts[lo:lo + NPC]

        pos = _greedy_windows(src_c, dst_c)
        psrc = pos[src_c]                        # permuted src row ids
        pwin = (psrc // NW).astype(np.int64)

        degp = np.concatenate([deg, np.full(NPAD - NPC, -1, dtype=deg.dtype)])
        nperm = np.argsort(-degp, kind="stable")

        cnt = np.zeros((NPC, W), dtype=np.int32)
        np.add.at(cnt, (dst_c, pwin), 1)
        cntp = np.vstack([cnt, np.zeros((NPAD - NPC, W), np.int32)])[nperm]
        tilemax = np.maximum(tilemax, cntp.reshape(T, P, W).max(axis=1))

        eorder = np.lexsort((pwin, dst_c))
        d_e = dst_c[eorder]
        ps_e = psrc[eorder]
        ew_e = ew_c[eorder]
        nstart = np.zeros(NPC + 1, dtype=np.int64)
        np.cumsum(np.bincount(d_e, minlength=NPC), out=nstart[1:])
        wstart = np.zeros((NPC, W + 1), dtype=np.int64)
        for w in range(W):
            wstart[:, w + 1] = wstart[:, w] + cnt[:, w]
        percore.append((nperm, cnt, nstart, wstart, ps_e, ew_e, pos))

    chunks, SUMS, plan, fsizes, foffs, FTOT = _make_schedule(tilemax)

    cores = []
    for c in range(C):
        nperm, cnt, nstart, wstart, ps_e, ew_e, pos = percore[c]
        idx_all = np.zeros((P, FTOT), dtype=np.int16)
        ew_all = np.zeros((P, SUMS), dtype=ml_dtypes.bfloat16)
        for ci, (t0, t1, Sw, coff) in enumerate(chunks):
            ntile = t1 - t0
            # per-window ELL entries [ntile*128, S] in slot-major column
            # order: window-region col j = s*ntile + t
            roff = 0
            wdat = {}
            for w in range(W):
                S = int(Sw[w])
                if S == 0:
                    continue
                nodes = nperm[t0 * P:t1 * P]           # [ntile*128]
                real = nodes < NPC
                nn = np.minimum(nodes, NPC - 1)
                c_w = np.where(real, cnt[nn, w], 0)
                base_e = nstart[nn] + wstart[nn, w]
                k = np.arange(S)
                valid = k[None, :] < c_w[:, None]
                e = base_e[:, None] + np.minimum(
                    k[None, :], np.maximum(c_w[:, None] - 1, 0))
                ividx = np.where(valid, ps_e[e] - w * NW, 0).astype(np.int16)
                ivew = np.where(valid, ew_e[e], 0.0)
                # [ntile, 128, S] -> columns [S, ntile] of 128 (slot-major)
                ividx = ividx.reshape(ntile, P, S).transpose(2, 0, 1)
                ivew = ivew.reshape(ntile, P, S).transpose(2, 0, 1)
                ividx = ividx.reshape(S * ntile, P)    # [cols, 128]
                ivew = ivew.reshape(S * ntile, P)
                wdat[w] = ividx
                cols = S * ntile
                ew_all[:, coff + roff:coff + roff + cols] = \
                    ivew.T.astype(ml_dtypes.bfloat16)
                roff += cols
            # idx streams per gather, banded by queue
            for (w, c0, gc, q, boff) in plan[ci]:
                ividx = wdat[w]
                flat = ividx[c0:c0 + gc].reshape(gc * P)   # list order
                wrapped = flat.reshape(gc * P // 16, 16).T  # [16, F/...]
                f0 = foffs[ci] + boff
                F = gc * 8
                idx_all[32 * q:32 * q + 16, f0:f0 + F] = wrapped
                idx_all[32 * q + 16:32 * q + 32, f0:f0 + F] = wrapped

        cores.append((idx_all, ew_all, nperm, pos))
    return chunks, SUMS, plan, fsizes, foffs, FTOT, cores


def _emit_gather(gp, mybir, out_ap, in_ap, idxs_ap, num_idxs, queue_num):
    """dma_gather emit without the 256B elem restriction (128B elems)."""
    from concourse.bass import exact_div
    stride_bytes = 2 * D * 2                         # 256B row stride
    return gp.add_instruction(
        mybir.InstDMAGatherAnt(
            name=gp.bass.get_next_instruction_name(),
            ins=[*gp.lower_ap_dma(in_ap, for_custom_bir_dma=True),
                 gp.lower_ap(idxs_ap),
                 gp.lower_val_access(gp.to_reg(num_idxs))],
            outs=[gp.lower_ap(out_ap)],
            transpose=False,
            num_idxs=num_idxs,
            elem_size=D,
            stride_bytes_256=exact_div(stride_bytes, 256),
            gen_mode=0,
            single_packet=False,
            queue_num=queue_num,
            sbuf_tokens_per_rank=0,
            sbuf_free_dim_per_rank=0,
            sbuf_free_dim_pad_per_rank=0,
            sbuf_byte_offset=0,
        ))


def _build_program(chunks, SUMS, plan, fsizes, foffs, FTOT):
    import concourse.bass as bass
    import concourse.mybir as mybir
    import concourse.tile as tile
    from concourse import bacc
    from concourse.masks import make_identity

    f32 = mybir.dt.float32
    bf16 = mybir.dt.bfloat16
    i16 = mybir.dt.int16

    nc = bacc.Bacc("TRN2", target_bir_lowering=False, debug=False,
                   num_devices=C, dynamic_dma_scratch_size=49152,
                   num_swdge_queues=NQ)

    featb = nc.dram_tensor("featb", [N_NODES, 2 * D], bf16,
                           kind="ExternalInput").ap()
    idx_d = nc.dram_tensor("idx", [P, FTOT], i16, kind="ExternalInput").ap()
    ew_d = nc.dram_tensor("ew", [P, SUMS], bf16, kind="ExternalInput").ap()
    wt_d = nc.dram_tensor("wt", [D, D], f32, kind="ExternalInput").ap()
    brep_d = nc.dram_tensor("brep", [P, D], f32, kind="ExternalInput").ap()
    out_d = nc.dram_tensor("out", [NPAD, D], f32, kind="ExternalOutput").ap()

    with tile.TileContext(nc) as tc:
        with (
            tc.tile_pool(name="const", bufs=1) as cpool,
            tc.tile_pool(name="sb", bufs=3) as pool,
            tc.tile_pool(name="ps", bufs=2, space="PSUM") as pspool,
        ):
            W_sb = cpool.tile([D, D], f32)
            nc.sync.dma_start(W_sb[:], wt_d[:])
            brep_sb = cpool.tile([P, D], f32)
            nc.sync.dma_start(brep_sb[:], brep_d[:])
            ident = cpool.tile([P, P], f32)
            make_identity(nc, ident[:])

            for ci, (t0, t1, Sw, coff) in enumerate(chunks):
                ntile = t1 - t0
                ccols = ntile * int(Sw.sum())
                fsz = max(fsizes[ci], 8)

                idx_sb = pool.tile([P, fsz], i16, tag="idx")
                nc.sync.dma_start(idx_sb[:],
                                  idx_d[:, foffs[ci]:foffs[ci] + fsz])
                ew_sb = pool.tile([P, ccols], bf16, tag="ew")
                nc.sync.dma_start(ew_sb[:], ew_d[:, coff:coff + ccols])

                msgs = pool.tile([P, ccols * D], bf16, tag="msgs")
                # window region offsets within chunk
                woff = {}
                roff = 0
                for w in range(W):
                    S = int(Sw[w])
                    if S == 0:
                        continue
                    woff[w] = roff
                    roff += ntile * S
                for (w, c0, gc, q, boff) in plan[ci]:
                    o0 = woff[w] + c0
                    fw = featb[w * NW:(w + 1) * NW, :]
                    _emit_gather(
                        nc.gpsimd, mybir,
                        out_ap=msgs[:, o0 * D:(o0 + gc) * D].rearrange(
                            "p (c d) -> p c d", d=D),
                        in_ap=fw,
                        idxs_ap=idx_sb[:, boff:boff + gc * 8],
                        num_idxs=gc * P,
                        queue_num=q,
                    )

                # in-place multiply by edge weight
                m3 = msgs[:].rearrange("p (s d) -> p s d", d=D)
                ewb = ew_sb[:].unsqueeze(2).to_broadcast([P, ccols, D])
                nc.vector.tensor_tensor(out=m3, in0=m3, in1=ewb,
                                        op=mybir.AluOpType.mult)

                # per-window contiguous slot trees; level 1 casts to f32
                h1s = {w: max(int(Sw[w]) // 2, 1)
                       for w in range(W) if int(Sw[w]) > 0}
                acc = pool.tile([P, ntile * sum(h1s.values()) * D], f32,
                                tag="acc")
                hviews = []
                aoff = 0
                for w in range(W):
                    S = int(Sw[w])
                    if S == 0:
                        continue
                    h1 = h1s[w]
                    nd = ntile * D
                    mw = msgs[:, woff[w] * D:(woff[w] + ntile * S) * D]
                    aw = acc[:, aoff * D:(aoff + ntile * h1) * D]
                    if S == 1:
                        nc.vector.tensor_scalar(
                            out=aw[:, 0:nd], in0=mw[:, 0:nd],
                            scalar1=1.0, scalar2=None,
                            op0=mybir.AluOpType.mult)
                    else:
                        nc.vector.tensor_tensor(
                            out=aw[:, 0:h1 * nd], in0=mw[:, 0:h1 * nd],
                            in1=mw[:, h1 * nd:2 * h1 * nd],
                            op=mybir.AluOpType.add)
                        if S % 2 == 1:
                            nc.vector.tensor_tensor(
                                out=aw[:, 0:nd], in0=aw[:, 0:nd],
                                in1=mw[:, (S - 1) * nd:S * nd],
                                op=mybir.AluOpType.add)
                    cur = h1
                    while cur > 1:
                        h = cur // 2
                        nc.vector.tensor_tensor(
                            out=aw[:, 0:h * nd], in0=aw[:, 0:h * nd],
                            in1=aw[:, (cur - h) * nd:cur * nd],
                            op=mybir.AluOpType.add)
                        cur = cur - h
                    hviews.append(aw[:, 0:nd])
                    aoff += ntile * h1

                houtc = pool.tile([P, ntile * D], f32, tag="houtc")
                if len(hviews) == 1:
                    nc.vector.tensor_scalar(
                        out=houtc[:], in0=hviews[0], scalar1=1.0,
                        scalar2=None, op0=mybir.AluOpType.mult)
                else:
                    nc.vector.tensor_tensor(out=houtc[:], in0=hviews[0],
                                            in1=hviews[1],
                                            op=mybir.AluOpType.add)
                    for hv in hviews[2:]:
                        nc.vector.tensor_tensor(out=houtc[:], in0=houtc[:],
                                                in1=hv,
                                                op=mybir.AluOpType.add)

                outc = pool.tile([P, ntile * D], f32, tag="outc")
                for j in range(ntile):
                    hj = houtc[:, j * D:(j + 1) * D]
                    pt = pspool.tile([D, P], f32, tag="pt")
                    nc.tensor.transpose(pt[:], hj, ident[:])
                    ht = pool.tile([D, P], f32, tag="ht")
                    nc.scalar.copy(ht[:], pt[:])
                    po = pspool.tile([P, D], f32, tag="po")
                    nc.tensor.matmul(po[:], lhsT=ht[:], rhs=W_sb[:],
                                     start=True, stop=True)
                    nc.vector.tensor_tensor(
                        out=outc[:, j * D:(j + 1) * D], in0=po[:],
                        in1=brep_sb[:], op=mybir.AluOpType.add)

                od = out_d[t0 * P:t1 * P, :].rearrange("(t p) d -> p t d", p=P)
                nc.scalar.dma_start(od, outc[:].rearrange("p (t d) -> p t d",
                                                          d=D))

    nc.compile()
    return nc


def _prepare(feat, edge_w, src, dst, weight, bias):
    feat = np.asarray(feat, dtype=np.float32)
    edge_w = np.asarray(edge_w, dtype=np.float32)
    src = np.asarray(src, dtype=np.int32)
    dst = np.asarray(dst, dtype=np.int32)
    weight = np.asarray(weight, dtype=np.float32)
    bias = np.asarray(bias, dtype=np.float32)

    chunks, SUMS, plan, fsizes, foffs, FTOT, cores = _build_tables(
        src, dst, edge_w)
    brep = np.ascontiguousarray(np.broadcast_to(bias, (P, D))).astype(
        np.float32)

    feat_pad = np.zeros((N_NODES, 2 * D), dtype=ml_dtypes.bfloat16)
    feat_pad[:, :D] = feat.astype(ml_dtypes.bfloat16)

    in_maps = []
    perms = []
    for c in range(C):
        idx_all, ew_all, nperm, pos = cores[c]
        fp = np.empty_like(feat_pad)
        fp[pos] = feat_pad                       # row r of fp = feat[inv]
        in_maps.append({
            "featb": fp,
            "idx": idx_all,
            "ew": ew_all,
            "wt": weight,
            "brep": brep,
        })
        perms.append(nperm)
    return chunks, SUMS, plan, fsizes, foffs, FTOT, in_maps, perms


def kernel(feat, edge_w, src, dst, weight, bias, _trace=False):
    (chunks, SUMS, plan, fsizes, foffs, FTOT, in_maps,
     perms) = _prepare(feat, edge_w, src, dst, weight, bias)

    key = tuple((t0, t1, tuple(Sw)) for (t0, t1, Sw, _) in chunks)
    if key not in _cache:
        _cache[key] = _build_program(chunks, SUMS, plan, fsizes, foffs, FTOT)
    nc = _cache[key]

    from concourse.bass_utils import run_bass_kernel_spmd
    res = run_bass_kernel_spmd(nc, in_maps, core_ids=list(range(C)),
                               trace=_trace)
    out = np.empty((N_NODES, D), dtype=np.float32)
    for c in range(C):
        o = np.asarray(res.results[c]["out"])
        nperm = perms[c]
        real = nperm < NPC
        out[c * NPC + nperm[real]] = o[np.nonzero(real)[0]]
    if _trace:
        kernel.last_results = res
    return out
